# revision 10
# baseline (speedup 1.0000x reference)
"""TRN2 Bass kernel for nn_CIE_48052094108098 (sparse_attention).

Model (S=2048, B=4, D=512, H=8 -> HH=4 heads/module, DH=128):
  gates = sigmoid(MLP([mean(x[:1024]), mean(x[1024:]), |diff|]))   (per batch)
  xn = LayerNorm(x)
  homo-MHA: same-half block attention, v gated by gates[:,0]
  het-MHA:  cross-half block attention, v gated by gates[:,1]
  y = x + homo + het;  out = y + FFN(y)   (GELU exact)

Sharding: 8 cores = (batch b in 0..3) x (query half p in 0..1). Each core
computes the full output for its (b, half-p) rows; the block masks make each
attention dense over 1024-wide blocks. Zero cross-core communication; the
host reorders columns per core so the kernel is uniform SPMD.

v3: fp8(e4m3) DoubleRow matmuls (2 contraction tiles per pass) for LN stats,
QKV projections, attn-weight @ V, softmax denominators, out-projection and
both FFN layers; scores in fp8 at bf16 speed (contraction=128). All DRAM
tensors partition-major; small consts merged into one DMA; x DMA split by
partition groups across queues. PE warmed with junk matmuls during the DMA
window. ACT uses one table for the whole pre-FFN phase (sqrt early, gater
sigmoid computed via exp, relu on DVE). Attention 1/den multiplies and the
y fp8 casts run on gpsimd. v-gate/v-bias commute through attention (host
sends Wo@bv; gate folds into 1/den). rel-err budget 2e-2; fp8 ~1.5e-2.
"""
import sys

sys.path.insert(0, "/opt/trn_rl_repo")

import numpy as np

S, B, D = 2048, 4, 512
SH = S // 2          # 1024 (half)
HH, DH = 4, 128      # heads per module, head dim
HID = 128            # gater hidden
FF = 4 * D           # 2048
NCORE = 8
ND = D // 128        # 4 feature tiles
NFF = FF // 128      # 16
NT = SH // 128       # 8 t-tiles per kv half
NSQ = SH // 512      # 2 query s-chunks
NCONST = 51          # merged const columns

_CACHED = {}


def build_nc():
    import concourse.mybir as mybir
    import concourse.tile as tile
    from concourse import bacc

    F32 = mybir.dt.float32
    F32R = mybir.dt.float32r
    BF16 = mybir.dt.bfloat16
    F8 = mybir.dt.float8e4
    ACTF = mybir.ActivationFunctionType
    ALU = mybir.AluOpType
    DR = mybir.MatmulPerfMode.DoubleRow

    nc = bacc.Bacc("TRN2", target_bir_lowering=False, debug=False,
                   num_devices=NCORE)

    # ---- DRAM parameters (all partition-major: [128, ...]) ----
    dp = nc.declare_dram_parameter
    x8_d = dp("x8", [128, ND * S], F8, isOutput=False)       # cols [q half | o half]
    xq32_d = dp("xq32", [128, ND * SH], F32, isOutput=False)
    gw1T_d = dp("gw1T", [128, 12 * HID], BF16, isOutput=False)
    cst_d = dp("cst", [128, NCONST], F32, isOutput=False)
    ngb2_d = dp("ngb2", [1, 2], F32, isOutput=False)
    onesr_d = dp("onesr", [1, 128], F32, isOutput=False)
    wq_d, wk_d, wv_d, wo_d = {}, {}, {}, {}
    for m in ("h", "t"):
        wq_d[m] = dp(f"wq_{m}", [128, ND * D], F8, isOutput=False)
        wk_d[m] = dp(f"wk_{m}", [128, ND * D], F8, isOutput=False)
        wv_d[m] = dp(f"wv_{m}", [128, ND * D], F8, isOutput=False)
        wo_d[m] = dp(f"wo_{m}", [128, ND * D], F8, isOutput=False)
    w1_d = dp("w1", [128, ND * FF], F8, isOutput=False)
    w2_d = dp("w2", [128, NFF * D], F8, isOutput=False)
    zT_d = dp("zT", [128, ND * SH], F32, isOutput=True)

    INV_SQRT_DH = float(1.0 / np.sqrt(DH))

    lp = nc.allow_low_precision("fp8/f32r intermediates; 2e-2 rel-err budget")
    lp.__enter__()
    with tile.TileContext(nc, pool_alloc_mode="queue") as tc:
        const = tc.alloc_tile_pool(name="const", bufs=1)
        big = tc.alloc_tile_pool(name="big", bufs=1)
        ppt = tc.alloc_tile_pool(name="ppt", bufs=4)
        pp8 = tc.alloc_tile_pool(name="pp8", bufs=2)
        pbc = tc.alloc_tile_pool(name="pbc", bufs=3)
        psmall = tc.alloc_tile_pool(name="psmall", bufs=16)
        pz = tc.alloc_tile_pool(name="pz", bufs=4)
        psum = tc.alloc_tile_pool(name="psum", bufs=2, space="PSUM")

        def ps_big2(nm):       # [128,1024] two-bank psum ring
            return psum.tile([128, 1024], F32, name=nm, tag="big2", bufs=2)

        def ps_av2(nm, shape=(128, 1024)):
            return psum.tile(list(shape), F32, name=nm, tag="av2", bufs=2)

        # ---------- on-device constants (no DMA descriptors burned) ------
        ones8_t = const.tile([128, 2, 16], F8)
        nc.vector.memset(ones8_t, 1.0)
        ones8 = ones8_t[:, :, 0:1]
        onesr_t = const.tile([1, 128], F32R)
        onesr = onesr_t[:, :]
        eps_t = const.tile([1, 1], F32)
        nc.vector.memset(eps_t, 1e-5)

        # ---------- input DMAs ------------------------------------------
        # x8 q-half first, split into 32-partition slices to land on many
        # queues at once (descriptor processing is the DMA bottleneck)
        x8 = big.tile([128, ND, S], F8, tag="t_x8")
        for d in range(ND):
            for g in range(4):
                nc.sync.dma_start(out=x8[32 * g:32 * (g + 1), d, 0:SH],
                                  in_=x8_d[32 * g:32 * (g + 1), d * S:d * S + SH])
        for d in range(ND):
            for g in range(2):
                nc.sync.dma_start(out=x8[64 * g:64 * (g + 1), d, SH:S],
                                  in_=x8_d[64 * g:64 * (g + 1), d * S + SH:(d + 1) * S])
        cst_t = const.tile([128, NCONST], F32)
        nc.sync.dma_start(out=cst_t, in_=cst_d[:, :])
        ngb2_t = const.tile([1, 2], F32)
        nc.sync.dma_start(out=ngb2_t, in_=ngb2_d[:, :])
        nc.sync.dma_start(out=onesr_t, in_=onesr_d[:, :].bitcast(F32R))
        bqk_t = {"h": cst_t[:, 0:8], "t": cst_t[:, 8:16]}
        wbv_t = {"h": cst_t[:, 16:20], "t": cst_t[:, 20:24]}
        bo_t = cst_t[:, 24:28]
        b1_t = cst_t[:, 28:44]
        b2_t = cst_t[:, 44:48]
        gb1_t = cst_t[:, 48:49]
        gw2T_t = cst_t[:, 49:51]
        gw1T_t = const.tile([128, 12, HID], BF16)
        nc.sync.dma_start(out=gw1T_t, in_=gw1T_d[:, :].rearrange("p (n f) -> p n f", n=12))
        w8 = {}
        for m in ("h", "t"):
            for nm, dd in (("q", wq_d), ("k", wk_d), ("v", wv_d)):
                t = big.tile([128, ND, D], F8, name=f"w{nm}8_{m}", tag=f"t_w{nm}{m}")
                nc.sync.dma_start(out=t, in_=dd[m][:, :].rearrange("p (n f) -> p n f", n=ND))
                w8[nm, m] = t
        wo8 = {}
        for m in ("h", "t"):
            wo8[m] = big.tile([128, ND, D], F8, name=f"wo8_{m}", tag=f"t_wo{m}")
            nc.sync.dma_start(out=wo8[m], in_=wo_d[m][:, :].rearrange("p (n f) -> p n f", n=ND))
        w18 = big.tile([128, ND, FF], F8, tag="t_w1")
        nc.sync.dma_start(out=w18, in_=w1_d[:, :].rearrange("p (n f) -> p n f", n=ND))
        w28 = big.tile([128, NFF, D], F8, tag="t_w2")
        nc.sync.dma_start(out=w28, in_=w2_d[:, :].rearrange("p (n f) -> p n f", n=NFF))
        xq32 = big.tile([128, ND, SH], F32, tag="t_xq32")
        for d in range(ND):
            nc.sync.dma_start(out=xq32[:, d, :], in_=xq32_d[:, d * SH:(d + 1) * SH])

        # ---------- PE warmup: dep-free junk matmuls ramp the pstate -----
        warm_ps = ps_big2("warm_ps")
        for i in range(40):
            nc.tensor.matmul(warm_ps[0:1, 0:16], ones8, ones8_t[:, :, :],
                             start=True, stop=True, perf_mode=DR)

        # ---------- LN stats: q half (fp8 DoubleRow over feature pairs) --
        xn8 = big.tile([128, ND, S], F8, tag="t_xn8")
        sq8 = big.tile([128, ND, SH], F8, name="xsq8", tag="t_sq")

        def stats_mm(dst, src, c0):
            for c in range(2):
                for j in range(2):
                    nc.tensor.matmul(dst[:, c * 512:(c + 1) * 512], ones8,
                                     src[:, 2 * j:2 * j + 2, c0 + c * 512:c0 + (c + 1) * 512],
                                     start=(j == 0), stop=(j == 1), perf_mode=DR)

        sum_q = ps_av2("sum_q", (1, 1024))
        stats_mm(sum_q, x8, 0)
        for d in range(ND):            # ACT: squares q
            nc.scalar.activation(sq8[:, d, :], x8[:, d, 0:SH], ACTF.Square)
        ssq_q = ps_av2("ssq_q", (1, 1024))
        stats_mm(ssq_q, sq8, 0)

        # row math: 1/std and mean/std (short chain of [1,1024] ops)
        def row_math(sum_ps, ssq_ps, tag):
            mean = psmall.tile([1, 1024], F32, name=f"mean{tag}", tag="row", bufs=4)
            nc.vector.tensor_scalar_mul(mean, sum_ps, 1.0 / D)
            msq = psmall.tile([1, 1024], F32, name=f"msq{tag}", tag="row", bufs=4)
            nc.vector.tensor_mul(msq, mean, mean)
            var = psmall.tile([1, 1024], F32, name=f"var{tag}", tag="row", bufs=4)
            nc.vector.scalar_tensor_tensor(out=var, in0=ssq_ps, scalar=1.0 / D, in1=msq,
                                           op0=ALU.mult, op1=ALU.subtract)  # ssq/D - mean^2
            std = psmall.tile([1, 1024], F32, name=f"std{tag}", tag="row", bufs=4)
            nc.scalar.activation(std, var, ACTF.Sqrt, bias=eps_t)
            rstd32 = psmall.tile([1, 1024], F32, name=f"rstd32{tag}", tag="row", bufs=4)
            nc.vector.reciprocal_approx_fast(rstd32, std)
            rstd = psmall.tile([1, 1024], F32R, name=f"rstd{tag}", tag="row", bufs=4)
            nc.vector.tensor_copy(rstd, rstd32)
            mr = psmall.tile([1, 1024], F32R, name=f"mr{tag}", tag="row", bufs=4)
            nc.vector.tensor_mul(mr, mean, rstd32)
            return rstd, mr

        rstd_q, mr_q = row_math(sum_q, ssq_q, "q")

        def bcast_mm(rstd, mr, mk):
            rB = mk("rstdB")
            mB = mk("mrB")
            for c in range(2):
                nc.tensor.matmul(rB[:, c * 512:(c + 1) * 512], onesr,
                                 rstd[:, c * 512:(c + 1) * 512],
                                 start=True, stop=True)
                nc.tensor.matmul(mB[:, c * 512:(c + 1) * 512], onesr,
                                 mr[:, c * 512:(c + 1) * 512],
                                 start=True, stop=True)
            return rB, mB

        rstdB_q, mrB_q = bcast_mm(rstd_q, mr_q, lambda nm: ps_av2(nm + "_q"))
        sum_o = psum.tile([1, 1024], F32, name="sum_o", tag="big2", bufs=2)
        stats_mm(sum_o, x8, SH)
        # a little more junk keeps the PE from napping while DVE applies LN
        for i in range(40):
            nc.tensor.matmul(warm_ps[0:1, 16:32], ones8, ones8_t[:, :, :],
                             start=True, stop=True, perf_mode=DR)

        # apply q half, chunked at 512 so head-0 projections start early
        for c in range(2):
            for d in range(ND):
                t1 = ppt.tile([128, 512], F32, name=f"lnt0_{d}{c}", tag="pth", bufs=2)
                nc.vector.tensor_mul(t1, x8[:, d, c * 512:(c + 1) * 512],
                                     rstdB_q[:, c * 512:(c + 1) * 512])
                nc.vector.tensor_sub(xn8[:, d, c * 512:(c + 1) * 512], t1,
                                     mrB_q[:, c * 512:(c + 1) * 512])
        # ACT: squares o (same table; before the gater copies)
        for d in range(ND):
            nc.scalar.activation(sq8[:, d, :], x8[:, d, SH:S], ACTF.Square)

        # ---------- gater (ACT copies early; sigmoid via exp table) ------
        gates = {}
        bo_eff_box = {}
        g_in = []

        def gater_means():
            fq_l, fo_l = [], []
            for d in range(ND):
                junk = ppt.tile([128, 1024], F32, name=f"gjq{d}", tag="pt", bufs=2)
                fq = psmall.tile([128, 1], F32, name=f"fq{d}", tag="gsm")
                nc.scalar.activation(junk, x8[:, d, 0:SH], ACTF.Copy, accum_out=fq)
                fq_m = psmall.tile([128, 1], BF16, name=f"fqm{d}", tag="gsm")
                nc.vector.tensor_scalar_mul(fq_m, fq, 1.0 / SH)
                fq_l.append(fq_m)
            for d in range(ND):
                junk = ppt.tile([128, 1024], F32, name=f"gjo{d}", tag="pt", bufs=2)
                fo = psmall.tile([128, 1], F32, name=f"fo{d}", tag="gsm")
                nc.scalar.activation(junk, x8[:, d, SH:S], ACTF.Copy, accum_out=fo)
                fo_m = psmall.tile([128, 1], BF16, name=f"fom{d}", tag="gsm")
                nc.vector.tensor_scalar_mul(fo_m, fo, 1.0 / SH)
                fo_l.append(fo_m)
            for d in range(ND):
                ad = psmall.tile([128, 1], BF16, name=f"ad{d}", tag="gsm")
                nc.vector.tensor_sub(ad, fq_l[d], fo_l[d])
                ab = psmall.tile([128, 1], BF16, name=f"ab{d}", tag="gsm")
                nc.scalar.activation(ab, ad, ACTF.Abs)
                g_in.append(ab)
            g_in[:0] = fq_l + fo_l      # order: f_q tiles, f_o tiles, |diff|

        gater_means()

        bo1_box = {}

        def gater_tail():
            g1_psum = psum.tile([128, 1], F32, name="g1_psum", tag="big2", bufs=2)
            for i in range(12):
                nc.tensor.matmul(g1_psum, gw1T_t[:, i, :], g_in[i],
                                 start=(i == 0), stop=(i == 11))
            relu_t = psmall.tile([128, 1], F32, tag="gsm")
            nc.vector.tensor_scalar(out=relu_t, in0=g1_psum, scalar1=gb1_t,
                                    scalar2=0.0, op0=ALU.add, op1=ALU.max)
            for j, m in enumerate(("h", "t")):
                g2_psum = psum.tile([1, 1], F32, name=f"g2_psum{j}", tag="big2", bufs=2)
                nc.tensor.matmul(g2_psum, gw2T_t[:, j:j + 1], relu_t, start=True, stop=True)
                # sigmoid(z+gb2) = 1/(1+exp(-z-gb2)) -- stays on the exp table
                eg = psmall.tile([1, 1], F32, name=f"eg{j}", tag="gsm")
                nc.scalar.activation(eg, g2_psum, ACTF.Exp, scale=-1.0,
                                     bias=ngb2_t[:, j:j + 1])
                ep1 = psmall.tile([1, 1], F32, name=f"ep1{j}", tag="gsm")
                nc.vector.tensor_scalar_add(ep1, eg, 1.0)
                gate = psmall.tile([1, 1], F32, name=f"gate{j}", tag="gsm")
                nc.vector.reciprocal_approx_fast(gate, ep1)
                gates[m] = gate
                g128 = pbc.tile([128, 1], F32, name=f"g128_{j}", tag="g128", bufs=2)
                nc.gpsimd.partition_broadcast(g128, gate)
                if m == "h":
                    bo1 = psmall.tile([128, ND], F32, name="bo1", tag="boe", bufs=2)
                    nc.vector.scalar_tensor_tensor(out=bo1, in0=wbv_t["h"], scalar=g128,
                                                   in1=bo_t, op0=ALU.mult, op1=ALU.add)
                    bo1_box["v"] = bo1
                else:
                    bo_eff = psmall.tile([128, ND], F32, name="bo_eff", tag="boe", bufs=2)
                    nc.vector.scalar_tensor_tensor(out=bo_eff, in0=wbv_t["t"], scalar=g128,
                                                   in1=bo1_box["v"], op0=ALU.mult, op1=ALU.add)
                    bo_eff_box["v"] = bo_eff

        # deferred o-half LN pieces (issued via module-h head hooks)
        obox = {}

        def o_stage1():        # at h0: gater tail + o-half ssq + row math
            gater_tail()
            ssq_o = ps_big2("ssq_o")
            stats_mm(ssq_o[0:1, :], sq8, 0)
            obox["rm"] = row_math(sum_o, ssq_o[0:1, :], "o")

        def o_stage2():        # at h1: o-half broadcasts + LN apply
            rstd_o, mr_o = obox["rm"]
            rstdB_o, mrB_o = bcast_mm(rstd_o, mr_o, lambda nm: ps_big2(nm + "_o"))
            for d in range(ND):
                t1 = ppt.tile([128, 1024], F32, name=f"lnt1_{d}", tag="pt", bufs=2)
                nc.vector.tensor_mul(t1, x8[:, d, SH:S], rstdB_o)
                nc.vector.tensor_sub(xn8[:, d, SH:S], t1, mrB_o)

        # ---------- per-module: V/K/Q proj (fp8 DR), per-head attention --
        ao8 = {}
        ao8["h"] = big.tile([128, HH, SH], F8, name="ao8_h", tag="t_aoh")

        def run_module(m, prelude=None, hooks={}):
            kv0 = 0 if m == "h" else SH
            wqm, wkm, wvm = w8["q", m], w8["k", m], w8["v", m]
            qt = big.tile([128, HH, SH], F8, name=f"qt_{m}", tag=f"t_qt{m}")
            kt = big.tile([128, HH, SH], F8, name=f"kt_{m}", tag=f"t_kt{m}")
            v8 = big.tile([128, NT, D], F8, name=f"v_{m}", tag=f"t_v{m}")
            aom = ao8[m]

            def kqproj(w, dst, ft, boff, src0):
                kp = ps_big2(f"kqp_{m}{boff}{ft}")
                for c in range(NSQ):
                    for j in range(2):
                        nc.tensor.matmul(kp[:, c * 512:(c + 1) * 512],
                                         w[:, 2 * j:2 * j + 2, ft * 128:(ft + 1) * 128],
                                         xn8[:, 2 * j:2 * j + 2, src0 + c * 512:src0 + (c + 1) * 512],
                                         start=(j == 0), stop=(j == 1), perf_mode=DR)
                nc.vector.tensor_scalar_add(dst[:, ft, :], kp,
                                            bqk_t[m][:, boff + ft:boff + ft + 1])

            def vproj(tt):
                vp = ps_big2(f"vp_{m}{tt}")
                for j in range(2):
                    nc.tensor.matmul(vp[:, 0:512],
                                     xn8[:, 2 * j:2 * j + 2, kv0 + tt * 128:kv0 + (tt + 1) * 128],
                                     wvm[:, 2 * j:2 * j + 2, :],
                                     start=(j == 0), stop=(j == 1), perf_mode=DR)
                nc.vector.tensor_copy(v8[:, tt, :], vp[:, 0:512])

            def avden(av_ps, den_ps, p8, h, u):
                for sq in range(NSQ):
                    nc.tensor.matmul(av_ps[:, sq * 512:(sq + 1) * 512],
                                     v8[:, 2 * u:2 * u + 2, h * 128:(h + 1) * 128],
                                     p8[:, 2 * u:2 * u + 2, sq * 512:(sq + 1) * 512],
                                     start=(u == 0), stop=(u == NT // 2 - 1), perf_mode=DR)
                for sq in range(NSQ):
                    nc.tensor.matmul(den_ps[:, sq * 512:(sq + 1) * 512], ones8,
                                     p8[:, 2 * u:2 * u + 2, sq * 512:(sq + 1) * 512],
                                     start=(u == 0), stop=(u == NT // 2 - 1), perf_mode=DR)

            if m == "h":
                for tt in range(NT // 2):
                    vproj(tt)
                kqproj(wkm, kt, 0, 4, kv0)
                kqproj(wqm, qt, 0, 0, 0)
                for tt in range(NT // 2, NT):
                    vproj(tt)
            else:
                kqproj(wkm, kt, 0, 4, kv0)
                kqproj(wqm, qt, 0, 0, 0)
                for tt in range(NT):
                    vproj(tt)
            if prelude is not None:
                prelude()

            def finish(h, aou, rden):
                denB = pbc.tile([128, 1024], F32, name=f"denB_{m}{h}", tag="bc", bufs=2)
                nc.gpsimd.partition_broadcast(denB, rden)
                nc.gpsimd.tensor_mul(aom[:, h, :], aou, denB)

            pend = None
            for h in range(HH):
                if h + 1 < HH:
                    kqproj(wkm, kt, h + 1, 4, kv0)
                    kqproj(wqm, qt, h + 1, 0, 0)
                if h in hooks:
                    hooks[h]()
                if pend is not None:
                    finish(*pend)
                    pend = None
                av_ps = ps_av2(f"av_{m}{h}")
                den_ps = ps_av2(f"den_{m}{h}", (1, 1024))
                p8 = pp8.tile([128, NT, SH], F8, name=f"p8_{m}{h}", tag="p8", bufs=2)
                for u in range(NT // 2):
                    for tt in (2 * u, 2 * u + 1):
                        sp = ps_big2(f"sp_{m}{h}{tt}")
                        for sq in range(NSQ):
                            nc.tensor.matmul(sp[:, sq * 512:(sq + 1) * 512],
                                             kt[:, h, tt * 128:(tt + 1) * 128],
                                             qt[:, h, sq * 512:(sq + 1) * 512],
                                             start=True, stop=True)
                        nc.scalar.activation(p8[:, tt, :], sp, ACTF.Exp, scale=INV_SQRT_DH)
                    if u > 0:
                        avden(av_ps, den_ps, p8, h, u - 1)
                avden(av_ps, den_ps, p8, h, NT // 2 - 1)
                aou = ppt.tile([128, 1024], F32, name=f"aou_{m}{h}", tag="aou", bufs=2)
                nc.vector.tensor_copy(aou, av_ps)
                rden32 = psmall.tile([1, 1024], F32, name=f"rden32_{m}{h}", tag="row", bufs=4)
                nc.vector.reciprocal_approx_fast(rden32, den_ps)
                rden = psmall.tile([1, 1024], F32, name=f"rden_{m}{h}", tag="row", bufs=4)
                nc.vector.tensor_scalar_mul(rden, rden32, gates[m])
                pend = (h, aou, rden)
            return lambda: finish(*pend)

        fin_h = run_module("h", hooks={0: o_stage1, 1: o_stage2})
        ao8["t"] = big.tile([128, HH, SH], F8, name="ao8_t", tag="t_aot")
        fin_t = run_module("t", prelude=fin_h)
        fin_t()

        # ---------- out-proj (fp8 DR, both modules into one psum) --------
        y32 = big.tile([128, ND, SH], F32, tag="t_y32")
        y8 = big.tile([128, ND, SH], F8, tag="t_y8")
        for ft in range(ND):
            op = ps_big2(f"op_{ft}")
            for mi, m in enumerate(("h", "t")):
                for j in range(2):
                    for sq in range(NSQ):
                        nc.tensor.matmul(op[:, sq * 512:(sq + 1) * 512],
                                         wo8[m][:, 2 * j:2 * j + 2, ft * 128:(ft + 1) * 128],
                                         ao8[m][:, 2 * j:2 * j + 2, sq * 512:(sq + 1) * 512],
                                         start=(mi == 0 and j == 0), stop=(mi == 1 and j == 1),
                                         perf_mode=DR)
            nc.vector.scalar_tensor_tensor(
                out=y32[:, ft, :], in0=op, scalar=bo_eff_box["v"][:, ft:ft + 1],
                in1=xq32[:, ft, :], op0=ALU.add, op1=ALU.add)
            nc.gpsimd.tensor_copy(y8[:, ft, :], y32[:, ft, :])

        # ---------- FFN (fp8 DR both layers) ----------
        for sq in range(NSQ):
            z_ps = [ps_av2(f"z2_{sq}{i}") for i in range(2)]
            h8_t = {}

            def ffn1(ff, sq=sq, h8_t=h8_t):
                hp = ps_big2(f"hp_{sq}{ff}")
                for j in range(2):
                    nc.tensor.matmul(hp[:, 0:512],
                                     w18[:, 2 * j:2 * j + 2, ff * 128:(ff + 1) * 128],
                                     y8[:, 2 * j:2 * j + 2, sq * 512:(sq + 1) * 512],
                                     start=(j == 0), stop=(j == 1), perf_mode=DR)
                w = ff // 2
                if ff % 2 == 0:
                    h8_t[w] = pp8.tile([128, 2, 512], F8, name=f"h8_{sq}{w}",
                                       tag="h8", bufs=3)
                nc.scalar.activation(h8_t[w][:, ff % 2, :], hp[:, 0:512], ACTF.Gelu,
                                     bias=b1_t[:, ff:ff + 1])

            def ffn2(w, sq=sq, h8_t=h8_t, z_ps=z_ps):
                for ot in range(ND):
                    nc.tensor.matmul(z_ps[ot // 2][:, (ot % 2) * 512:(ot % 2 + 1) * 512],
                                     w28[:, 2 * w:2 * w + 2, ot * 128:(ot + 1) * 128],
                                     h8_t[w], start=(w == 0), stop=(w == NFF // 2 - 1),
                                     perf_mode=DR)

            for w in range(NFF // 2):
                ffn1(2 * w)
                ffn1(2 * w + 1)
                if w > 0:
                    ffn2(w - 1)
            ffn2(NFF // 2 - 1)
            for ot in range(ND):
                z_t = pz.tile([128, 512], F32, name=f"z_t{sq}{ot}", tag="z")
                nc.vector.scalar_tensor_tensor(
                    out=z_t, in0=z_ps[ot // 2][:, (ot % 2) * 512:(ot % 2 + 1) * 512],
                    scalar=b2_t[:, ot:ot + 1],
                    in1=y32[:, ot, sq * 512:(sq + 1) * 512],
                    op0=ALU.add, op1=ALU.add)
                nc.sync.dma_start(out=zT_d[:, ot * SH + sq * 512:ot * SH + (sq + 1) * 512],
                                  in_=z_t)

        psum.release()
        pz.release()
        psmall.release()
        pbc.release()
        pp8.release()
        ppt.release()
        big.release()
        const.release()

    lp.__exit__(None, None, None)
    nc.finalize()
    return nc


def _pm(a, n):
    """[n*128, cols] -> partition-major [128, n*cols]."""
    a = np.ascontiguousarray(a)
    return np.ascontiguousarray(
        a.reshape(n, 128, -1).transpose(1, 0, 2).reshape(128, -1))


def _prep_inputs(sequence, g_w1, g_b1, g_w2, g_b2, ln_g, ln_b,
                 homo_in_w, homo_in_b, homo_out_w, homo_out_b,
                 het_in_w, het_in_b, het_out_w, het_out_b,
                 ffn_w1, ffn_b1, ffn_w2, ffn_b2):
    import ml_dtypes
    bf16 = ml_dtypes.bfloat16
    f8 = ml_dtypes.float8_e4m3
    f32 = np.float32
    cc = np.ascontiguousarray

    def rev(v, n):     # [n*128] bias -> [128, n] column layout
        return np.asarray(v, f32).reshape(n, 128).T

    shared = {}
    ln_g = np.asarray(ln_g, f32)
    ln_b = np.asarray(ln_b, f32)
    cst = np.zeros((128, NCONST), f32)
    for mi, (m, in_w, in_b, out_w) in enumerate(
            (("h", homo_in_w, homo_in_b, homo_out_w),
             ("t", het_in_w, het_in_b, het_out_w))):
        in_w = np.asarray(in_w, f32)
        in_b = np.asarray(in_b, f32)
        out_w = np.asarray(out_w, f32)
        wq, wk, wv = in_w[0:D], in_w[D:2 * D], in_w[2 * D:3 * D]
        # fold LN affine into the projections: W' = W*diag(g), b' = b + W@ln_b
        shared[f"wq_{m}"] = _pm((wq * ln_g).T.astype(f8), ND)
        shared[f"wk_{m}"] = _pm((wk * ln_g).T.astype(f8), ND)
        shared[f"wv_{m}"] = _pm((wv * ln_g).T.astype(f8), ND)
        bqk = in_b[0:2 * D].copy()
        bqk[0:D] += wq @ ln_b
        bqk[D:2 * D] += wk @ ln_b
        cst[:, 8 * mi:8 * mi + 8] = rev(bqk, 8)
        cst[:, 16 + 4 * mi:20 + 4 * mi] = rev(out_w @ (in_b[2 * D:3 * D] + wv @ ln_b), ND)
        shared[f"wo_{m}"] = _pm(out_w.T.astype(f8), ND)
    cst[:, 24:28] = rev(np.asarray(homo_out_b, f32) + np.asarray(het_out_b, f32), ND)
    cst[:, 28:44] = rev(ffn_b1, NFF)
    cst[:, 44:48] = rev(ffn_b2, ND)
    cst[:, 48] = np.asarray(g_b1, f32)
    cst[:, 49:51] = np.asarray(g_w2, f32).T
    shared["cst"] = cc(cst)
    shared["ngb2"] = cc(-np.asarray(g_b2, f32).reshape(1, 2))
    shared["onesr"] = np.ones((1, 128), f32)
    shared["w1"] = _pm(np.asarray(ffn_w1, f32).T.astype(f8), ND)
    shared["w2"] = _pm(np.asarray(ffn_w2, f32).T.astype(f8), NFF)

    g_w1 = np.asarray(g_w1, f32)
    gw1T = g_w1.T.astype(bf16)                       # [1536, HID]: [f_s|f_b|diff]
    gw1T_swap = np.concatenate([gw1T[D:2 * D], gw1T[0:D], gw1T[2 * D:]], axis=0)

    seq = np.asarray(sequence, f32)
    in_maps = []
    for core in range(NCORE):
        b, p = core // 2, core % 2
        xb = seq[:, b, :]                            # [S, D]
        xq = xb[p * SH:(p + 1) * SH]
        xo = xb[(1 - p) * SH:(2 - p) * SH]
        xT32 = np.concatenate([xq, xo], axis=0).T    # [D, S]
        mm = dict(shared)
        mm["x8"] = _pm(xT32.astype(f8), ND)
        mm["xq32"] = _pm(xT32[:, 0:SH], ND)
        mm["gw1T"] = _pm(gw1T if p == 0 else gw1T_swap, 12)
        in_maps.append(mm)
    return in_maps


def kernel(**inputs):
    from concourse.bass_utils import run_bass_kernel_spmd

    if "nc" not in _CACHED:
        _CACHED["nc"] = build_nc()
    nc = _CACHED["nc"]

    in_maps = _prep_inputs(**{k: np.asarray(v) for k, v in inputs.items()})
    core_ids = list(range(NCORE))
    res = run_bass_kernel_spmd(nc, in_maps, core_ids)

    out = np.empty((S, B, D), np.float32)
    for core in range(NCORE):
        b, p = core // 2, core % 2
        z = res.results[core]["zT"].reshape(128, ND, SH)
        out[p * SH:(p + 1) * SH, b, :] = z.transpose(1, 0, 2).reshape(D, SH).T
    return out
